# revision 1
# baseline (speedup 1.0000x reference)
"""Trainium2 Bass kernel for a decoder block (LN -> MHA -> LN -> FFN).

Sharding: heads across the 8 cores for attention (2 heads/core), tokens
across cores for dense/LN2/FFN (512 tokens/core), connected by an
AllToAll of the (unnormalized ctx, softmax denom) in bf16 — split into
one collective per batch so the first overlaps batch-1 attention.

All matmuls run in bf16 with fp32 PSUM accumulation; layernorm stats,
softmax input, and residuals stay fp32.  gamma1/beta1 are folded into
the QKV weights/biases on the host, gamma2/beta2 into fc_w and a fc
output bias; the true (gamma,beta)-applied xn / hn needed for the
residual connections are computed on-chip for the core's own token
shard only.
"""

import numpy as np
import ml_dtypes

B, S, D = 2, 2048, 1024
H, DEP = 16, 64
NT = B * S            # 4096 flattened tokens
NCORES = 8
HPC = H // NCORES     # 2 heads per core
TPC = NT // NCORES    # 512 tokens per core
QM = 512              # q-macro / token-macro size
KT = 128              # k-tile size
NEG = -1.0e9

_cache = {}


def _build_program():
    from contextlib import ExitStack
    import concourse.bacc as bacc
    import concourse.tile as tile
    import concourse.mybir as mybir
    from concourse.masks import make_identity

    dt = mybir.dt
    AF = mybir.ActivationFunctionType
    OP = mybir.AluOpType

    nc = bacc.Bacc("TRN2", target_bir_lowering=False, debug=False,
                   num_devices=NCORES)

    def din(name, shape, dtype=dt.float32):
        return nc.dram_tensor(name, shape, dtype, kind="ExternalInput").ap()

    x_full = din("x_full", [NT, D])
    x_shard = din("x_shard", [TPC, D])
    wqt = din("wqt", [D, 128], dt.bfloat16)
    wkt = din("wkt", [D, 128], dt.bfloat16)
    wvt = din("wvt", [D, 128], dt.bfloat16)
    qb_i = din("qb", [128, 1])
    kb_i = din("kb", [128, 1])
    vb_i = din("vb", [128, 1])
    g1b_i = din("g1b", [128, D])
    b1b_i = din("b1b", [128, D])
    g2b_i = din("g2b", [128, D])
    b2b_i = din("b2b", [128, D])
    dense_wt = din("dense_wt", [D, D], dt.bfloat16)
    fc_wt = din("fc_wt", [32, D, 128], dt.bfloat16)
    fcb_i = din("fcb", [128, 32])
    proj_wt = din("proj_wt", [4 * D, D], dt.bfloat16)
    mask_i = din("mask_diag", [4, 128, QM])
    out_sh = nc.dram_tensor("out_shard", [TPC, D], dt.float32,
                            kind="ExternalOutput").ap()

    VAR_SCALE = float(D) / float(D - 1)   # ddof=1 correction
    EPS = 1e-6

    with tile.TileContext(nc) as tc:
        with ExitStack() as es0:
            consts = es0.enter_context(tc.tile_pool(name="consts", bufs=1))
            dram = es0.enter_context(
                tc.tile_pool(name="dram", bufs=1, space="DRAM"))
            ident_bf = consts.tile([128, 128], dt.bfloat16)
            make_identity(nc, ident_bf)
            ones_bf = consts.tile([1, 64], dt.bfloat16)
            nc.vector.memset(ones_bf, 1.0)
            qb = consts.tile([128, 1], dt.float32)
            kb = consts.tile([128, 1], dt.float32)
            vb = consts.tile([128, 1], dt.float32)
            nc.sync.dma_start(out=qb, in_=qb_i)
            nc.sync.dma_start(out=kb, in_=kb_i)
            nc.sync.dma_start(out=vb, in_=vb_i)

            xnsh_pool = es0.enter_context(tc.tile_pool(name="xnsh", bufs=1))
            xn_sh = xnsh_pool.tile([128, 4, D], dt.float32)

            # batch-mixed shards: core c owns 256 tokens of each batch
            # (batch b, macro c//2, half c%2), so both A2As carry only
            # real data and land in disjoint token ranges of ctxT.
            a2a_in = [dram.tile([NCORES, HPC, DEP, QM // 2], dt.bfloat16,
                                name=f"a2a_in{bb}") for bb in range(2)]
            a2a_out = [dram.tile([NCORES, HPC, DEP, QM // 2], dt.bfloat16,
                                 name=f"a2a_out{bb}") for bb in range(2)]

            # ------- LN1 + QKV interleaved, then attention per batch -------
            with ExitStack() as es1:
                P = lambda *a, **k: es1.enter_context(tc.tile_pool(*a, **k))
                xt_pool = P(name="xt", bufs=3)
                st_pool = P(name="stats", bufs=3)
                xnT_pool = P(name="xnT", bufs=1)
                qkT_pool = P(name="qkT", bufs=1)
                v_pool = P(name="vtok", bufs=1)
                wq_pool = P(name="wq", bufs=1)
                ps_tr = P(name="ps_tr", bufs=1, space="PSUM")
                ps_sc = P(name="ps_sc", bufs=3, space="PSUM")
                ps_qk = P(name="ps_qk", bufs=1, space="PSUM")
                pr_pool = P(name="probs", bufs=22)
                psctx = P(name="psctx", bufs=2, space="PSUM")
                ps_bc = P(name="ps_bc", bufs=1, space="PSUM")
                vstage_pool = P(name="vstage", bufs=3)
                a2a_pool = P(name="a2asb", bufs=3)
                c1_pool = P(name="c1", bufs=1)

                masks = c1_pool.tile([128, 4, QM], dt.float32)
                for i in range(4):
                    nc.sync.dma_start(out=masks[:, i, :], in_=mask_i[i])

                xn_T = xnT_pool.tile([128, 8, NT], dt.bfloat16)
                q_T = qkT_pool.tile([128, NT], dt.bfloat16)
                k_T = qkT_pool.tile([128, NT], dt.bfloat16)
                vtok = v_pool.tile([128, 32, 130], dt.bfloat16)
                nc.vector.memset(vtok[:, :, 64:65], 1.0)
                nc.vector.memset(vtok[:, :, 129:130], 1.0)

                wq_sb = wq_pool.tile([128, 8, 128], dt.bfloat16)
                wk_sb = wq_pool.tile([128, 8, 128], dt.bfloat16)
                wv_sb = wq_pool.tile([128, 8, 128], dt.bfloat16)
                nc.sync.dma_start(
                    out=wq_sb, in_=wqt.rearrange("(c p) m -> p c m", p=128))
                nc.sync.dma_start(
                    out=wk_sb, in_=wkt.rearrange("(c p) m -> p c m", p=128))
                nc.sync.dma_start(
                    out=wv_sb, in_=wvt.rearrange("(c p) m -> p c m", p=128))

                def ln_macro(src, base_row):
                    """Stats for 4 consecutive 128-row tiles; batched rstd.
                    Returns list of (x_t, mean_ap, rstd_ap)."""
                    mv4 = st_pool.tile([128, 4, 2], dt.float32, tag="mv4")
                    xts = []
                    for i in range(4):
                        x_t = xt_pool.tile([128, D], dt.float32, tag="xt", bufs=5)
                        r0 = base_row + 128 * i
                        nc.sync.dma_start(out=x_t, in_=src[r0:r0 + 128, :])
                        stats = st_pool.tile([128, 2, 6], dt.float32,
                                             tag="bnst")
                        nc.vector.bn_stats(out=stats[:, 0, :],
                                           in_=x_t[:, 0:512])
                        nc.vector.bn_stats(out=stats[:, 1, :],
                                           in_=x_t[:, 512:1024])
                        nc.vector.bn_aggr(out=mv4[:, i, :], in_=stats)
                        xts.append(x_t)
                    rstd4 = st_pool.tile([128, 4], dt.float32, tag="rstd4")
                    nc.scalar.activation(out=rstd4, in_=mv4[:, :, 1],
                                         func=AF.Sqrt, scale=VAR_SCALE)
                    nc.vector.tensor_scalar_add(rstd4, rstd4, EPS)
                    nc.vector.reciprocal(out=rstd4, in_=rstd4)
                    return [(xts[i], mv4[:, i, 0:1], rstd4[:, i:i + 1])
                            for i in range(4)]

                def qkv_macro(m):
                    tok = slice(QM * m, QM * (m + 1))
                    for w_sb, bias, dst in ((wq_sb, qb, q_T), (wk_sb, kb, k_T)):
                        ps = ps_qk.tile([128, QM], dt.float32, tag="qk")
                        for kc in range(8):
                            nc.tensor.matmul(ps, w_sb[:, kc, :],
                                             xn_T[:, kc, tok],
                                             start=(kc == 0), stop=(kc == 7))
                        nc.vector.tensor_scalar_add(dst[:, tok], ps, bias)
                    ps = ps_qk.tile([128, QM], dt.float32, tag="qk")
                    for kc in range(8):
                        nc.tensor.matmul(ps, wv_sb[:, kc, :], xn_T[:, kc, tok],
                                         start=(kc == 0), stop=(kc == 7))
                    vst = vstage_pool.tile([128, QM], dt.bfloat16, tag="vst")
                    nc.vector.tensor_scalar_add(vst, ps, vb)
                    for half in range(2):
                        pt = ps_tr.tile([128, 2, 128], dt.bfloat16, tag="xtr")
                        for s2 in range(2):
                            s = 2 * half + s2
                            nc.tensor.transpose(
                                pt[:, s2, :], vst[:, 128 * s:128 * (s + 1)],
                                ident_bf)
                        for s2 in range(2):
                            kt_idx = 4 * m + 2 * half + s2
                            nc.vector.tensor_copy(out=vtok[:, kt_idx, 0:64],
                                                  in_=pt[:, s2, 0:64])
                            nc.vector.tensor_copy(out=vtok[:, kt_idx, 65:129],
                                                  in_=pt[:, s2, 64:128])

                def attention_macro(b, m):
                    q0 = 2048 * b + QM * m
                    nkt = 4 * m + 4
                    probs = []
                    for j in range(nkt):
                        rel = j - 4 * m
                        lo = 128 * rel if rel > 0 else 0
                        ks = slice(2048 * b + KT * j,
                                   2048 * b + KT * (j + 1))
                        pair = []
                        for h in range(2):
                            hp = slice(64 * h, 64 * (h + 1))
                            ps = ps_sc.tile([128, QM], dt.float32,
                                            tag="sc")
                            nc.tensor.matmul(
                                ps[:, lo:QM], k_T[hp, ks],
                                q_T[hp, q0 + lo:q0 + QM],
                                start=True, stop=True)
                            if rel >= 0:
                                nc.vector.tensor_add(
                                    ps[:, lo:QM], ps[:, lo:QM],
                                    masks[:, rel, lo:QM])
                            pb = pr_pool.tile([128, QM], dt.bfloat16,
                                              tag="pr")
                            if lo > 0:
                                nc.vector.memset(pb[:, 0:lo], 0.0)
                            nc.scalar.activation(out=pb[:, lo:QM],
                                                 in_=ps[:, lo:QM],
                                                 func=AF.Exp, scale=0.125)
                            pair.append(pb)
                        probs.append(pair)
                    for h in range(2):
                        pc = psctx.tile([65, QM], dt.float32, tag="ctx")
                        for j in range(nkt):
                            nc.tensor.matmul(
                                pc,
                                vtok[:, 16 * b + j, 65 * h:65 * (h + 1)],
                                probs[j][h],
                                start=(j == 0), stop=(j == nkt - 1))
                        r32 = a2a_pool.tile([1, QM], dt.float32,
                                            tag="r32")
                        nc.vector.reciprocal(out=r32, in_=pc[64:65, :])
                        rbf = a2a_pool.tile([1, QM], dt.bfloat16,
                                            tag="rbf")
                        nc.vector.tensor_copy(out=rbf, in_=r32)
                        pb2 = ps_bc.tile([64, QM], dt.float32, tag="bc")
                        nc.tensor.matmul(pb2, ones_bf, rbf,
                                         start=True, stop=True)
                        rb_sb = a2a_pool.tile([64, QM], dt.bfloat16,
                                              tag="rbsb")
                        nc.vector.tensor_copy(out=rb_sb, in_=pb2)
                        csb = a2a_pool.tile([64, QM], dt.bfloat16,
                                            tag="csb")
                        nc.vector.tensor_tensor(out=csb, in0=pc[0:64, :],
                                                in1=rb_sb, op=OP.mult)
                        for hf in range(2):
                            nc.sync.dma_start(
                                out=a2a_in[b][2 * m + hf, h],
                                in_=csb[:, 256 * hf:256 * (hf + 1)])

                for m in range(8):
                    for i, (x_t, mean, rstd) in enumerate(
                            ln_macro(x_full, QM * m)):
                        t = 4 * m + i
                        xnb = xt_pool.tile([128, D], dt.bfloat16, tag="xnb")
                        nc.vector.tensor_scalar(out=xnb, in0=x_t, scalar1=mean,
                                                scalar2=rstd, op0=OP.subtract,
                                                op1=OP.mult)
                        for half in range(2):
                            ps = ps_tr.tile([128, 4, 128], dt.bfloat16,
                                            tag="xtr")
                            for s2 in range(4):
                                kc = 4 * half + s2
                                nc.tensor.transpose(
                                    ps[:, s2, :],
                                    xnb[:, 128 * kc:128 * (kc + 1)], ident_bf)
                            # alternate copy-back engine: ACT / DVE
                            dst = xn_T[:, 4 * half:4 * half + 4,
                                       128 * t:128 * (t + 1)]
                            nc.scalar.copy(out=dst, in_=ps)
                    qkv_macro(m)
                    attention_macro(m // 4, m % 4)
                    if m == 3:
                        nc.gpsimd.collective_compute(
                            "AllToAll", mybir.AluOpType.bypass,
                            replica_groups=[list(range(NCORES))],
                            ins=[a2a_in[0].opt()], outs=[a2a_out[0].opt()],
                        )
                # true xn (gamma/beta applied, fp32) for own shard
                g1b = c1_pool.tile([128, D], dt.float32)
                b1b = c1_pool.tile([128, D], dt.float32)
                nc.sync.dma_start(out=g1b, in_=g1b_i)
                nc.sync.dma_start(out=b1b, in_=b1b_i)
                for i, (x_t, mean, rstd) in enumerate(ln_macro(x_shard, 0)):
                    xr = xt_pool.tile([128, D], dt.float32, tag="xr", bufs=2)
                    nc.vector.tensor_scalar(out=xr, in0=x_t, scalar1=mean,
                                            scalar2=rstd, op0=OP.subtract,
                                            op1=OP.mult)
                    nc.vector.tensor_mul(xr, xr, g1b)
                    nc.vector.tensor_add(xn_sh[:, i, :], xr, b1b)

                nc.gpsimd.collective_compute(
                    "AllToAll", mybir.AluOpType.bypass,
                    replica_groups=[list(range(NCORES))],
                    ins=[a2a_in[1].opt()], outs=[a2a_out[1].opt()],
                )

            # ---------------- dense, LN2, FFN --------------
            with ExitStack() as es2:
                P = lambda *a, **k: es2.enter_context(tc.tile_pool(*a, **k))
                h_pool = P(name="hh", bufs=1)
                st2_pool = P(name="st2", bufs=4)
                hnT_pool = P(name="hnT", bufs=1)
                g1_pool = P(name="g1sb", bufs=1)
                fc_pool = P(name="fcst", bufs=3)
                psd = P(name="psd", bufs=2, space="PSUM")
                pse = P(name="pse", bufs=2, space="PSUM")
                out_pool = P(name="outsb", bufs=3)
                c2_pool = P(name="c2", bufs=1)

                es2a = es2.enter_context(ExitStack())
                g2b = c2_pool.tile([128, D], dt.float32)
                b2b = c2_pool.tile([128, D], dt.float32)
                fcb = c2_pool.tile([128, 32], dt.float32)
                nc.sync.dma_start(out=g2b, in_=g2b_i)
                nc.sync.dma_start(out=b2b, in_=b2b_i)
                nc.sync.dma_start(out=fcb, in_=fcb_i)

                ctxT_pool = es2a.enter_context(
                    tc.tile_pool(name="ctxT", bufs=1))
                dense_sb = ctxT_pool.tile([128, 8, D], dt.bfloat16)
                nc.sync.dma_start(
                    out=dense_sb,
                    in_=dense_wt.rearrange("(c p) m -> p c m", p=128))
                ctxT = ctxT_pool.tile([128, 8, TPC], dt.bfloat16)
                resh = lambda t: t.rearrange("c h d q -> (c h d) q").rearrange(
                    "(k p) q -> p k q", p=128)
                nc.sync.dma_start(out=ctxT[:, :, 0:256], in_=resh(a2a_out[0]))
                nc.sync.dma_start(out=ctxT[:, :, 256:512],
                                  in_=resh(a2a_out[1]))

                # dense: token-major out; h = attn_out + dense_b + xn_sh
                h_t = h_pool.tile([128, 4, D], dt.float32)
                for ts in range(4):
                    tsl = slice(128 * ts, 128 * (ts + 1))
                    for dh in range(2):
                        dsl = slice(512 * dh, 512 * (dh + 1))
                        ps = psd.tile([128, QM], dt.float32, tag="dn")
                        for kc in range(8):
                            nc.tensor.matmul(ps, ctxT[:, kc, tsl],
                                             dense_sb[:, kc, dsl],
                                             start=(kc == 0), stop=(kc == 7))
                        nc.vector.tensor_add(h_t[:, ts, dsl], ps,
                                             xn_sh[:, ts, dsl])

                es2a.close()
                prj_pool = es2.enter_context(tc.tile_pool(name="prst", bufs=2))

                # LN2 -> hnraw (bf16, transposed) + true hn (fp32)
                hn_true = h_pool.tile([128, 4, D], dt.float32)
                hnT = hnT_pool.tile([128, 8, TPC], dt.bfloat16)
                mv4b = st2_pool.tile([128, 4, 2], dt.float32, tag="mv4b")
                for t in range(4):
                    stats = st2_pool.tile([128, 2, 6], dt.float32, tag="bnst2")
                    nc.vector.bn_stats(out=stats[:, 0, :], in_=h_t[:, t, 0:512])
                    nc.vector.bn_stats(out=stats[:, 1, :],
                                       in_=h_t[:, t, 512:1024])
                    nc.vector.bn_aggr(out=mv4b[:, t, :], in_=stats)
                rstd4b = st2_pool.tile([128, 4], dt.float32, tag="rstd4b")
                nc.scalar.activation(out=rstd4b, in_=mv4b[:, :, 1],
                                     func=AF.Sqrt, scale=VAR_SCALE)
                nc.vector.tensor_scalar_add(rstd4b, rstd4b, EPS)
                nc.vector.reciprocal(out=rstd4b, in_=rstd4b)
                for t in range(4):
                    hr = st2_pool.tile([128, D], dt.float32, tag="hr")
                    nc.vector.tensor_scalar(out=hr, in0=h_t[:, t, :],
                                            scalar1=mv4b[:, t, 0:1],
                                            scalar2=rstd4b[:, t:t + 1],
                                            op0=OP.subtract, op1=OP.mult)
                    nc.vector.tensor_mul(hn_true[:, t, :], hr, g2b)
                    nc.vector.tensor_add(hn_true[:, t, :], hn_true[:, t, :],
                                         b2b)
                    hrb = st2_pool.tile([128, D], dt.bfloat16, tag="hrb")
                    nc.scalar.copy(out=hrb, in_=hr)
                    for half in range(2):
                        pt = pse.tile([128, 4, 128], dt.bfloat16, tag="tr2")
                        for s2 in range(4):
                            kc = 4 * half + s2
                            nc.tensor.transpose(
                                pt[:, s2, :], hrb[:, 128 * kc:128 * (kc + 1)],
                                ident_bf)
                        dst = hnT[:, 4 * half:4 * half + 4,
                                  128 * t:128 * (t + 1)]
                        if (t + half) % 2 == 0:
                            nc.scalar.copy(out=dst, in_=pt)
                        else:
                            nc.vector.tensor_copy(out=dst, in_=pt)

                # FFN fc + gelu -> g1 (feature-major)
                g1 = g1_pool.tile([128, 32, TPC], dt.bfloat16)
                for ht in range(32):
                    fcw = fc_pool.tile([128, 8, 128], dt.bfloat16, tag="fcw")
                    nc.sync.dma_start(
                        out=fcw,
                        in_=fc_wt[ht].rearrange("(c p) m -> p c m", p=128))
                    ps = psd.tile([128, TPC], dt.float32, tag="fc")
                    for kc in range(8):
                        nc.tensor.matmul(ps, fcw[:, kc, :], hnT[:, kc, :],
                                         start=(kc == 0), stop=(kc == 7))
                    nc.scalar.activation(out=g1[:, ht, :], in_=ps,
                                         func=AF.Gelu,
                                         bias=fcb[:, ht:ht + 1], scale=1.0)

                # FFN proj: token-major out; out = hn_true + ff
                for dh in range(2):
                    dsl = slice(512 * dh, 512 * (dh + 1))
                    pw = prj_pool.tile([128, 32, QM], dt.bfloat16, tag="pw")
                    nc.sync.dma_start(
                        out=pw,
                        in_=proj_wt[:, dsl].rearrange("(c p) m -> p c m",
                                                      p=128))
                    for ts in range(4):
                        tsl = slice(128 * ts, 128 * (ts + 1))
                        ps = pse.tile([128, QM], dt.float32, tag="pj")
                        for j in range(32):
                            nc.tensor.matmul(ps, g1[:, j, tsl], pw[:, j, :],
                                             start=(j == 0), stop=(j == 31))
                        osb = out_pool.tile([128, QM], dt.float32, tag="osb")
                        nc.vector.tensor_add(osb, ps, hn_true[:, ts, dsl])
                        nc.sync.dma_start(out=out_sh[tsl, dsl], in_=osb)

    nc.compile()
    return nc


def _np_reference(x, mask, wq_w, wq_b, wk_w, wk_b, wv_w, wv_b, dense_w,
                  dense_b, gamma1, beta1, gamma2, beta2, fc_w, proj_w):
    """Pure-numpy fallback for non-causal masks (never hit in practice)."""
    import math
    erf = np.vectorize(math.erf)

    def ln(x, g, b):
        mu = x.mean(-1, keepdims=True)
        sd = x.std(-1, ddof=1, keepdims=True)
        return g * ((x - mu) / (sd + 1e-6)) + b

    x = x.astype(np.float64)
    xn = ln(x, gamma1, beta1)
    q = (xn @ wq_w.T + wq_b).reshape(B, S, H, DEP).transpose(0, 2, 1, 3)
    k = (xn @ wk_w.T + wk_b).reshape(B, S, H, DEP).transpose(0, 2, 1, 3)
    v = (xn @ wv_w.T + wv_b).reshape(B, S, H, DEP).transpose(0, 2, 1, 3)
    sc = np.einsum("bhqd,bhkd->bhqk", q, k) / np.sqrt(DEP) + mask * -1e9
    sc = sc - sc.max(-1, keepdims=True)
    e = np.exp(sc)
    a = e / e.sum(-1, keepdims=True)
    ctx = np.einsum("bhqk,bhkd->bhqd", a, v).transpose(0, 2, 1, 3).reshape(
        B, S, D)
    h = xn + ctx @ dense_w.T + dense_b
    hn = ln(h, gamma2, beta2)
    t = hn @ fc_w.T
    g = 0.5 * t * (1.0 + erf(t / np.sqrt(2.0)))
    return (hn + g @ proj_w.T).astype(np.float32)


def kernel(**inputs):
    x = np.asarray(inputs["x"], np.float32)
    mask = np.asarray(inputs["mask"], np.float32)

    causal = np.array_equal(mask, np.triu(np.ones((S, S), np.float32), k=1))
    if not causal:
        return _np_reference(**{k: np.asarray(v, np.float64 if
                                              np.asarray(v).dtype != np.int32
                                              else np.int32)
                                for k, v in inputs.items()}).reshape(B, S, D)

    if "nc" not in _cache:
        _cache["nc"] = _build_program()
    nc = _cache["nc"]

    bf16 = ml_dtypes.bfloat16
    g1 = np.asarray(inputs["gamma1"], np.float32)
    b1 = np.asarray(inputs["beta1"], np.float32)
    g2 = np.asarray(inputs["gamma2"], np.float32)
    b2 = np.asarray(inputs["beta2"], np.float32)
    dense_w = np.asarray(inputs["dense_w"], np.float32)
    dense_b = np.asarray(inputs["dense_b"], np.float32)
    fc_w = np.asarray(inputs["fc_w"], np.float32)
    proj_w = np.asarray(inputs["proj_w"], np.float32)

    xf = x.reshape(NT, D)
    shard_rows = []
    for c in range(NCORES):
        base = 512 * (c // 2) + 256 * (c % 2)
        shard_rows.append(np.concatenate(
            [base + np.arange(256), 2048 + base + np.arange(256)]))
    bcast = lambda v: np.ascontiguousarray(
        np.broadcast_to(v.astype(np.float32), (128, D)))

    # causal diagonal-block additive mask, scores_T orientation [k, q]
    md = np.zeros((4, 128, QM), np.float32)
    for i in range(4):
        kk = 128 * i + np.arange(128)[:, None]
        qq = np.arange(QM)[None, :]
        md[i][kk > qq] = NEG

    fc_eff = fc_w * g2[None, :]
    fcb = fc_w @ b2
    in_maps = []
    for c in range(NCORES):
        rows = slice(128 * c, 128 * (c + 1))
        im = {
            "x_full": xf,
            "x_shard": np.ascontiguousarray(xf[shard_rows[c]]),
            "g1b": bcast(g1), "b1b": bcast(b1 + dense_b),
            "g2b": bcast(g2), "b2b": bcast(b2),
            "dense_wt": dense_w.T.astype(bf16),
            "fc_wt": np.ascontiguousarray(
                fc_eff.T.reshape(D, 32, 128).transpose(1, 0, 2)).astype(bf16),
            "fcb": np.ascontiguousarray(fcb.reshape(32, 128).T),
            "proj_wt": proj_w.T.astype(bf16),
            "mask_diag": md,
        }
        for nm, w, bias in (("q", np.asarray(inputs["wq_w"], np.float32),
                             np.asarray(inputs["wq_b"], np.float32)),
                            ("k", np.asarray(inputs["wk_w"], np.float32),
                             np.asarray(inputs["wk_b"], np.float32)),
                            ("v", np.asarray(inputs["wv_w"], np.float32),
                             np.asarray(inputs["wv_b"], np.float32))):
            wslice = w[rows]                     # [128, D]
            im[f"w{nm}t"] = np.ascontiguousarray(
                (wslice * g1[None, :]).T).astype(bf16)
            im[f"{nm}b"] = (bias[rows] + wslice @ b1).reshape(128, 1)
        in_maps.append(im)

    global _last_in_maps
    _last_in_maps = in_maps
    from concourse import bass_utils
    res = bass_utils.run_bass_kernel_spmd(nc, in_maps,
                                          core_ids=list(range(NCORES)))
    out = np.empty((NT, D), np.float32)
    for c in range(NCORES):
        out[shard_rows[c]] = res.results[c]["out_shard"]
    return out.reshape(B, S, D)



# revision 62
# speedup vs baseline: 1.4211x; 1.4211x over previous
"""Trainium2 Bass kernel for a decoder block (LN -> MHA -> LN -> FFN).

Sharding: heads across the 8 cores for attention (2 heads/core), tokens
across cores for dense/LN2/FFN (512 tokens/core), connected by an
AllToAll of the (unnormalized ctx, softmax denom) in bf16 — split into
one collective per batch so the first overlaps batch-1 attention.

All matmuls run in bf16 with fp32 PSUM accumulation; layernorm stats,
softmax input, and residuals stay fp32.  gamma1/beta1 are folded into
the QKV weights/biases on the host, gamma2/beta2 into fc_w and a fc
output bias; the true (gamma,beta)-applied xn / hn needed for the
residual connections are computed on-chip for the core's own token
shard only.
"""

import os
import numpy as np
import ml_dtypes

B, S, D = 2, 2048, 1024
H, DEP = 16, 64
NT = B * S            # 4096 flattened tokens
NCORES = 8
HPC = H // NCORES     # 2 heads per core
TPC = NT // NCORES    # 512 tokens per core
QM = 512              # q-macro / token-macro size
KT = 128              # k-tile size
NEG = -1.0e9

_cache = {}


def _build_program():
    from contextlib import ExitStack
    import concourse.bacc as bacc
    import concourse.tile as tile
    import concourse.mybir as mybir
    from concourse.masks import make_identity

    dt = mybir.dt
    AF = mybir.ActivationFunctionType
    OP = mybir.AluOpType

    nc = bacc.Bacc("TRN2", target_bir_lowering=False, debug=False,
                   num_devices=NCORES)

    # The act-table pass greedily picks the FIRST table set containing each
    # activation func, so Ln->natural_log(5) / Exp->exp_and_others(0) thrash
    # a table reload (~1.3us) on every LN's rstd.  Steer both to set 6
    # (natural_log_exp_and_others, which really contains exp+ln+copy in
    # act_info.json) by hiding exp/ln from the earlier sets in the cached
    # table dict.  Set indices are unchanged, so the emitted
    # act_func_set_id still matches the compiler's act_info.json.
    from concourse.hw_specs import get_activation_tables
    if os.environ.get("KB_TABLES", "1") == "1":
        _tabs = get_activation_tables(nc.m.arch)
        for _name, _s in _tabs.items():
            if _name != "natural_log_exp_and_others":
                _s.discard(mybir.ActivationFunctionType.Exp)
                _s.discard(mybir.ActivationFunctionType.Ln)

    def din(name, shape, dtype=dt.float32):
        return nc.dram_tensor(name, shape, dtype, kind="ExternalInput").ap()

    x_full = din("x_full", [NT, D])
    x_shard = din("x_shard", [TPC, D])
    wqt = din("wqt", [128, 8, 128], dt.bfloat16)
    wkt = din("wkt", [128, 8, 128], dt.bfloat16)
    wvt = din("wvt", [128, 8, 128], dt.bfloat16)
    qb_i = din("qb", [128, 1])
    kb_i = din("kb", [128, 1])
    vb_i = din("vb", [128, 1])
    g1b_i = din("g1b", [128, D])
    b1b_i = din("b1b", [128, D])
    g2b_i = din("g2b", [128, D])
    b2b_i = din("b2b", [128, D])
    dense_wt = din("dense_wt", [D, D], dt.bfloat16)
    fc_wt = din("fc_wt", [32, 128, 8, 128], dt.bfloat16)
    fcb_i = din("fcb", [128, 32])
    proj_wt = din("proj_wt", [4 * D, D], dt.bfloat16)
    mask_i = din("mask_diag", [4, 128, QM])
    out_sh = nc.dram_tensor("out_shard", [TPC, D], dt.float32,
                            kind="ExternalOutput").ap()

    VAR_SCALE = float(D) / float(D - 1)   # ddof=1 correction
    EPS = 1e-6

    with tile.TileContext(nc) as tc:
        with ExitStack() as es0:
            consts = es0.enter_context(tc.tile_pool(name="consts", bufs=1))
            dram = es0.enter_context(
                tc.tile_pool(name="dram", bufs=1, space="DRAM"))
            ident_bf = consts.tile([128, 128], dt.bfloat16)
            make_identity(nc, ident_bf)
            ones_bf = consts.tile([1, 64], dt.bfloat16)
            nc.vector.memset(ones_bf, 1.0)
            qb = consts.tile([128, 1], dt.float32)
            kb = consts.tile([128, 1], dt.float32)
            vb = consts.tile([128, 1], dt.float32)
            nc.gpsimd.dma_start(out=qb, in_=qb_i)
            nc.gpsimd.dma_start(out=kb, in_=kb_i)
            nc.gpsimd.dma_start(out=vb, in_=vb_i)

            xnsh_pool = es0.enter_context(tc.tile_pool(name="xnsh", bufs=1))
            xn_sh = xnsh_pool.tile([128, 4, D], dt.float32)

            # phase-2 inputs that load during phase 1
            ctxT_pool = es0.enter_context(tc.tile_pool(name="ctxT", bufs=1))
            dense_sb = ctxT_pool.tile([128, 8, D], dt.bfloat16)
            ctxT = ctxT_pool.tile([128, 8, TPC], dt.bfloat16)
            resh_a2a = lambda t: t.rearrange(
                "c h d q -> (c h d) q").rearrange("(k p) q -> p k q", p=128)

            # batch-mixed shards: core c owns 256 tokens of each batch
            # (batch b, macro c//2, half c%2), so both A2As carry only
            # real data and land in disjoint token ranges of ctxT.
            a2a_in = [dram.tile([NCORES, HPC, DEP, QM // 2], dt.bfloat16,
                                name=f"a2a_in{bb}") for bb in range(2)]
            a2a_out = [dram.tile([NCORES, HPC, DEP, QM // 2], dt.bfloat16,
                                 name=f"a2a_out{bb}") for bb in range(2)]

            # ------- LN1 + QKV interleaved, then attention per batch -------
            with ExitStack() as es1:
                P = lambda *a, **k: es1.enter_context(tc.tile_pool(*a, **k))
                xt_pool = P(name="xt", bufs=3)
                st_pool = P(name="stats", bufs=3)
                xnT_pool = P(name="xnT", bufs=1)
                qkT_pool = P(name="qkT", bufs=1)
                v_pool = P(name="vtok", bufs=1)
                wq_pool = P(name="wq", bufs=1)
                ps_tr = P(name="ps_tr", bufs=2, space="PSUM")
                ps_sc = P(name="ps_sc", bufs=3, space="PSUM")
                ps_qk = P(name="ps_qk", bufs=1, space="PSUM")
                pr_pool = P(name="probs", bufs=22)
                psctx = P(name="psctx", bufs=2, space="PSUM")
                vstage_pool = P(name="vstage", bufs=3)
                a2a_pool = P(name="a2asb", bufs=3)
                c1_pool = P(name="c1", bufs=1)

                masks = c1_pool.tile([128, 4, QM], dt.bfloat16)
                nc.gpsimd.dma_start(out=masks,
                                    in_=mask_i.rearrange("i p q -> p i q"))

                # one batch's worth; macro m reuses column range (m%4)*QM
                # (batch-0 cols are dead after qkv_macro(3))
                xn_T = xnT_pool.tile([128, 8, NT // 2], dt.bfloat16)
                q_T = qkT_pool.tile([128, NT], dt.bfloat16)
                k_T = qkT_pool.tile([128, NT], dt.bfloat16)
                vtok = v_pool.tile([128, 32, 130], dt.bfloat16)
                nc.vector.memset(vtok[:, :, 64:65], 1.0)
                nc.vector.memset(vtok[:, :, 129:130], 1.0)

                wq_sb = wq_pool.tile([128, 8, 128], dt.bfloat16)
                wk_sb = wq_pool.tile([128, 8, 128], dt.bfloat16)
                wv_sb = wq_pool.tile([128, 8, 128], dt.bfloat16)

                def ln_macro(src, base_row):
                    """Stats for 4 consecutive 128-row tiles; per-tile rstd
                    so tile i's normalize starts without waiting on i+1..3.
                    rstd = (var*scale)^-0.5 via Exp(-0.5*Ln(.)): keeps all
                    phase-1 ACT funcs in the natural_log_exp table set (no
                    per-macro activation-table reloads). eps on std is 1e-6
                    relative — far below bf16 noise, so dropped."""
                    out = []
                    for i in range(4):
                        x_t = xt_pool.tile([128, D], dt.float32, tag="xt", bufs=8)
                        r0 = base_row + 128 * i
                        nc.sync.dma_start(out=x_t, in_=src[r0:r0 + 128, :])
                        stats = st_pool.tile([128, 2, 6], dt.float32,
                                             tag="bnst")
                        nc.vector.bn_stats(out=stats[:, 0, :],
                                           in_=x_t[:, 0:512])
                        nc.vector.bn_stats(out=stats[:, 1, :],
                                           in_=x_t[:, 512:1024])
                        mv = st_pool.tile([128, 2], dt.float32, tag="mv")
                        nc.vector.bn_aggr(out=mv, in_=stats)
                        rstd = st_pool.tile([128, 2], dt.float32, tag="rstd")
                        nc.scalar.activation(out=rstd[:, 0:1], in_=mv[:, 1:2],
                                             func=AF.Ln, scale=VAR_SCALE)
                        nc.scalar.activation(out=rstd[:, 1:2],
                                             in_=rstd[:, 0:1],
                                             func=AF.Exp, scale=-0.5)
                        out.append((x_t, mv[:, 0:1], rstd[:, 1:2]))
                    return out

                def qkv_macro(m):
                    tok = slice(QM * m, QM * (m + 1))          # global (q/k)
                    xtok = slice(QM * (m % 4), QM * (m % 4 + 1))  # xn_T local
                    for w_sb, bias, dst in ((wq_sb, qb, q_T), (wk_sb, kb, k_T)):
                        ps = ps_qk.tile([128, QM], dt.float32, tag="qk")
                        for kc in range(8):
                            nc.tensor.matmul(ps, w_sb[:, kc, :],
                                             xn_T[:, kc, xtok],
                                             start=(kc == 0), stop=(kc == 7))
                        nc.vector.tensor_scalar_add(dst[:, tok], ps, bias)
                    ps = ps_qk.tile([128, QM], dt.float32, tag="qk")
                    for kc in range(8):
                        nc.tensor.matmul(ps, wv_sb[:, kc, :], xn_T[:, kc, xtok],
                                         start=(kc == 0), stop=(kc == 7))
                    vst = vstage_pool.tile([128, QM], dt.bfloat16, tag="vst")
                    nc.vector.tensor_scalar_add(vst, ps, vb)
                    for half in range(2):
                        pt = ps_tr.tile([128, 2, 128], dt.bfloat16, tag="xtr")
                        for s2 in range(2):
                            s = 2 * half + s2
                            nc.tensor.transpose(
                                pt[:, s2, :], vst[:, 128 * s:128 * (s + 1)],
                                ident_bf)
                        for s2 in range(2):
                            kt_idx = 4 * m + 2 * half + s2
                            nc.vector.tensor_copy(out=vtok[:, kt_idx, 0:64],
                                                  in_=pt[:, s2, 0:64])
                            nc.vector.tensor_copy(out=vtok[:, kt_idx, 65:129],
                                                  in_=pt[:, s2, 64:128])

                def attention_macro(b, m):
                    q0 = 2048 * b + QM * m
                    nkt = 4 * m + 4
                    probs = []
                    for j in range(nkt):
                        rel = j - 4 * m
                        lo = 128 * rel if rel > 0 else 0
                        ks = slice(2048 * b + KT * j,
                                   2048 * b + KT * (j + 1))
                        pair = []
                        for h in range(2):
                            hp = slice(64 * h, 64 * (h + 1))
                            ps = ps_sc.tile([128, QM], dt.float32,
                                            tag="sc")
                            nc.tensor.matmul(
                                ps[:, lo:QM], k_T[hp, ks],
                                q_T[hp, q0 + lo:q0 + QM],
                                start=True, stop=(rel < 0))
                            if rel >= 0:
                                # accumulate the causal mask (-1e9 band,
                                # only cols [lo, lo+128) of the diag block)
                                # on PE: ident^T @ mask == mask
                                hi = min(lo + 128, QM)
                                nc.tensor.matmul(
                                    ps[:, lo:hi], ident_bf,
                                    masks[:, rel, lo:hi],
                                    start=False, stop=True,
                                    skip_group_check=True)
                            pb = pr_pool.tile([128, QM], dt.bfloat16,
                                              tag="pr")
                            nc.scalar.activation(out=pb[:, lo:QM],
                                                 in_=ps[:, lo:QM],
                                                 func=AF.Exp, scale=0.125)
                            pair.append(pb)
                        probs.append(pair)
                    csbs = []
                    for h in range(2):
                        pc = psctx.tile([65, QM], dt.float32, tag="ctx")
                        for j in range(nkt):
                            # diag tiles only contribute cols [lo:]; their
                            # probs are left ungarbage-collected below lo
                            rel = j - 4 * m
                            lo = 128 * rel if rel > 0 else 0
                            nc.tensor.matmul(
                                pc[:, lo:QM],
                                vtok[:, 16 * b + j, 65 * h:65 * (h + 1)],
                                probs[j][h][:, lo:QM],
                                start=(j == 0), stop=(j == nkt - 1))
                        den = a2a_pool.tile([1, QM], dt.float32,
                                            tag="r32")
                        nc.vector.tensor_copy(out=den, in_=pc[64:65, :])
                        rb_sb = a2a_pool.tile([64, QM], dt.float32,
                                              tag="rbsb")
                        nc.gpsimd.partition_broadcast(rb_sb, den)
                        csb = a2a_pool.tile([64, QM], dt.bfloat16,
                                            tag="csb")
                        nc.vector.tensor_tensor(out=csb, in0=pc[0:64, :],
                                                in1=rb_sb, op=OP.divide)
                        csbs.append(csb)
                    # stores after both heads' bcasts so the Pool queue
                    # doesn't stall a bcast behind a csb-blocked store;
                    # one merged DMA per head (partition-major src)
                    for h in range(2):
                        nc.gpsimd.dma_start(
                            out=a2a_in[b][2 * m:2 * m + 2, h].rearrange(
                                "hf d q -> d hf q"),
                            in_=csbs[h].rearrange("d (hf q) -> d hf q", hf=2))

                def ln_qkv(m):
                    for i, (x_t, mean, rstd) in enumerate(
                            ln_macro(x_full, QM * m)):
                        t = 4 * m + i
                        xnb = xt_pool.tile([128, D], dt.bfloat16, tag="xnb")
                        nc.vector.tensor_scalar(out=xnb[:, 0:512],
                                                in0=x_t[:, 0:512],
                                                scalar1=mean, scalar2=rstd,
                                                op0=OP.subtract, op1=OP.mult)
                        nc.vector.tensor_scalar(out=xnb[:, 512:1024],
                                                in0=x_t[:, 512:1024],
                                                scalar1=mean, scalar2=rstd,
                                                op0=OP.subtract, op1=OP.mult)
                        for half in range(2):
                            ps = ps_tr.tile([128, 4, 128], dt.bfloat16,
                                            tag="xtr")
                            for s2 in range(4):
                                kc = 4 * half + s2
                                nc.tensor.transpose(
                                    ps[:, s2, :],
                                    xnb[:, 128 * kc:128 * (kc + 1)], ident_bf)
                            tl = t % 16
                            dst = xn_T[:, 4 * half:4 * half + 4,
                                       128 * tl:128 * (tl + 1)]
                            if (t + half) % 4 == 0:
                                nc.scalar.copy(out=dst, in_=ps)
                            else:
                                nc.vector.tensor_copy(out=dst, in_=ps)
                    qkv_macro(m)

                ln_qkv(0)
                nc.sync.dma_start(out=wq_sb, in_=wqt)
                nc.sync.dma_start(out=wk_sb, in_=wkt)
                nc.sync.dma_start(out=wv_sb, in_=wvt)
                for m in range(8):
                    if m + 1 < 8:
                        ln_qkv(m + 1)
                    attention_macro(m // 4, m % 4)
                    if m == 5:
                        # own-shard true xn: emitted mid-batch-1 where DVE
                        # has slack (needed only by dense at ~220us)
                        g1b = c1_pool.tile([128, D], dt.float32)
                        b1b = c1_pool.tile([128, D], dt.float32)
                        nc.gpsimd.dma_start(out=g1b, in_=g1b_i)
                        nc.gpsimd.dma_start(out=b1b, in_=b1b_i)
                        for i, (x_t, mean, rstd) in enumerate(
                                ln_macro(x_shard, 0)):
                            xr = xt_pool.tile([128, D], dt.float32, tag="xr",
                                              bufs=2)
                            nc.gpsimd.scalar_tensor_tensor(
                                out=xr, in0=x_t, scalar=mean, in1=g1b,
                                op0=OP.subtract, op1=OP.mult)
                            nc.gpsimd.scalar_tensor_tensor(
                                out=xn_sh[:, i, :], in0=xr, scalar=rstd,
                                in1=b1b, op0=OP.mult, op1=OP.add)
                    if m == 3:
                        nc.gpsimd.collective_compute(
                            "AllToAll", mybir.AluOpType.bypass,
                            replica_groups=[list(range(NCORES))],
                            ins=[a2a_in[0].opt()], outs=[a2a_out[0].opt()],
                        )
                        nc.gpsimd.dma_start(out=ctxT[:, :, 0:256],
                                            in_=resh_a2a(a2a_out[0]))
                        nc.sync.dma_start(
                            out=dense_sb,
                            in_=dense_wt.rearrange("(c p) m -> p c m", p=128))

                nc.gpsimd.collective_compute(
                    "AllToAll", mybir.AluOpType.bypass,
                    replica_groups=[list(range(NCORES))],
                    ins=[a2a_in[1].opt()], outs=[a2a_out[1].opt()],
                )
                nc.gpsimd.dma_start(out=ctxT[:, :, 256:512],
                                    in_=resh_a2a(a2a_out[1]))

            # ------- dense, LN2, FFN: split in batch halves so half 0
            # (tokens 0:256, from a2a #0) runs while a2a #1 is in flight ----
            with ExitStack() as es2:
                P = lambda *a, **k: es2.enter_context(tc.tile_pool(*a, **k))
                h_pool = P(name="hh", bufs=1)
                st2_pool = P(name="st2", bufs=4)
                hnT_pool = P(name="hnT", bufs=1)
                g1_pool = P(name="g1sb", bufs=1)
                fcw_pool = P(name="fcw", bufs=1)
                psd = P(name="psd", bufs=2, space="PSUM")
                pse = P(name="pse", bufs=2, space="PSUM")
                psf = P(name="psf", bufs=2, space="PSUM")
                out_pool = P(name="outsb", bufs=3)
                c2_pool = P(name="c2", bufs=1)
                prj_pool = P(name="prst", bufs=2)

                g2b = c2_pool.tile([128, D], dt.bfloat16)
                b2b = c2_pool.tile([128, D], dt.bfloat16)
                fcb = c2_pool.tile([128, 32], dt.float32)
                nc.gpsimd.dma_start(out=g2b, in_=g2b_i)
                nc.gpsimd.dma_start(out=b2b, in_=b2b_i)
                nc.gpsimd.dma_start(out=fcb, in_=fcb_i)

                h_t = h_pool.tile([128, 4, D], dt.float32)
                hn_true = h_pool.tile([128, 4, D], dt.bfloat16)
                hnT = hnT_pool.tile([128, 8, TPC], dt.bfloat16)
                g1 = g1_pool.tile([128, 32, TPC], dt.bfloat16)
                mv4b = st2_pool.tile([128, 4, 2], dt.float32, tag="mv4b")
                rstd4b = st2_pool.tile([128, 4], dt.float32, tag="rstd4b")

                def dense_half(hb):
                    for ts in (2 * hb, 2 * hb + 1):
                        tsl = slice(128 * ts, 128 * (ts + 1))
                        for dh in range(2):
                            dsl = slice(512 * dh, 512 * (dh + 1))
                            ps = psd.tile([128, QM], dt.float32, tag="dn")
                            for kc in range(8):
                                nc.tensor.matmul(ps, ctxT[:, kc, tsl],
                                                 dense_sb[:, kc, dsl],
                                                 start=(kc == 0),
                                                 stop=(kc == 7))
                            nc.vector.tensor_add(h_t[:, ts, dsl], ps,
                                                 xn_sh[:, ts, dsl])

                def ln2_half(hb):
                    for t in (2 * hb, 2 * hb + 1):
                        stats = st2_pool.tile([128, 2, 6], dt.float32,
                                              tag="bnst2")
                        nc.vector.bn_stats(out=stats[:, 0, :],
                                           in_=h_t[:, t, 0:512])
                        nc.vector.bn_stats(out=stats[:, 1, :],
                                           in_=h_t[:, t, 512:1024])
                        nc.vector.bn_aggr(out=mv4b[:, t, :], in_=stats)
                    hsl = slice(2 * hb, 2 * hb + 2)
                    lnvb = st2_pool.tile([128, 2], dt.float32, tag="lnvb")
                    nc.scalar.activation(out=lnvb, in_=mv4b[:, hsl, 1],
                                         func=AF.Ln, scale=VAR_SCALE)
                    nc.scalar.activation(out=rstd4b[:, hsl], in_=lnvb,
                                         func=AF.Exp, scale=-0.5)
                    for t in (2 * hb, 2 * hb + 1):
                        hrb = st2_pool.tile([128, D], dt.bfloat16, tag="hrb")
                        negmr2 = st2_pool.tile([128, 1], dt.float32,
                                               tag="nmr2")
                        nc.vector.scalar_tensor_tensor(
                            out=negmr2, in0=mv4b[:, t, 0:1], scalar=-1.0,
                            in1=rstd4b[:, t:t + 1], op0=OP.mult, op1=OP.mult)
                        nc.vector.tensor_scalar(out=hrb[:, 0:512],
                                                in0=h_t[:, t, 0:512],
                                                scalar1=mv4b[:, t, 0:1],
                                                scalar2=rstd4b[:, t:t + 1],
                                                op0=OP.subtract, op1=OP.mult)
                        nc.scalar.activation(out=hrb[:, 512:1024],
                                             in_=h_t[:, t, 512:1024],
                                             func=AF.Identity,
                                             bias=negmr2,
                                             scale=rstd4b[:, t:t + 1])
                        nc.gpsimd.tensor_mul(hn_true[:, t, :], hrb, g2b)
                        nc.gpsimd.tensor_add(hn_true[:, t, :],
                                             hn_true[:, t, :], b2b)
                        for half in range(2):
                            pt = pse.tile([128, 4, 128], dt.bfloat16,
                                          tag="tr2")
                            for s2 in range(4):
                                kc = 4 * half + s2
                                nc.tensor.transpose(
                                    pt[:, s2, :],
                                    hrb[:, 128 * kc:128 * (kc + 1)], ident_bf)
                            dst = hnT[:, 4 * half:4 * half + 4,
                                      128 * t:128 * (t + 1)]
                            if (t + half) % 2 == 0:
                                nc.scalar.copy(out=dst, in_=pt)
                            else:
                                nc.vector.tensor_copy(out=dst, in_=pt)

                def fc_half(hb):
                    tok = slice(256 * hb, 256 * (hb + 1))
                    for ht in range(32):
                        fcw = fcw_pool.tile([128, 8, 128], dt.bfloat16,
                                            tag="fcw", bufs=4)
                        nc.sync.dma_start(out=fcw, in_=fc_wt[ht])
                        ps = psf.tile([128, TPC // 2], dt.float32, tag="fc")
                        for kc in range(8):
                            nc.tensor.matmul(ps, fcw[:, kc, :],
                                             hnT[:, kc, tok],
                                             start=(kc == 0), stop=(kc == 7))
                        nc.scalar.activation(out=g1[:, ht, tok], in_=ps,
                                             func=AF.Gelu,
                                             bias=fcb[:, ht:ht + 1],
                                             scale=1.0)

                def proj_half(hb):
                    for dc in range(4):
                        dsl = slice(256 * dc, 256 * (dc + 1))
                        pw = prj_pool.tile([128, 32, 256], dt.bfloat16,
                                           tag="pw")
                        nc.sync.dma_start(
                            out=pw,
                            in_=proj_wt[:, dsl].rearrange("(c p) m -> p c m",
                                                          p=128))
                        for ts in (2 * hb, 2 * hb + 1):
                            tsl = slice(128 * ts, 128 * (ts + 1))
                            ps = pse.tile([128, 256], dt.float32, tag="pj")
                            for j in range(32):
                                nc.tensor.matmul(ps, g1[:, j, tsl],
                                                 pw[:, j, :],
                                                 start=(j == 0),
                                                 stop=(j == 31))
                            osb = out_pool.tile([128, 256], dt.float32,
                                                tag="osb")
                            nc.vector.tensor_add(osb, ps,
                                                 hn_true[:, ts, dsl])
                            nc.gpsimd.dma_start(out=out_sh[tsl, dsl], in_=osb)

                for hb in range(2):
                    dense_half(hb)
                    ln2_half(hb)
                    fc_half(hb)
                    proj_half(hb)

    nc.compile()
    return nc


def _np_reference(x, mask, wq_w, wq_b, wk_w, wk_b, wv_w, wv_b, dense_w,
                  dense_b, gamma1, beta1, gamma2, beta2, fc_w, proj_w):
    """Pure-numpy fallback for non-causal masks (never hit in practice)."""
    import math
    erf = np.vectorize(math.erf)

    def ln(x, g, b):
        mu = x.mean(-1, keepdims=True)
        sd = x.std(-1, ddof=1, keepdims=True)
        return g * ((x - mu) / (sd + 1e-6)) + b

    x = x.astype(np.float64)
    xn = ln(x, gamma1, beta1)
    q = (xn @ wq_w.T + wq_b).reshape(B, S, H, DEP).transpose(0, 2, 1, 3)
    k = (xn @ wk_w.T + wk_b).reshape(B, S, H, DEP).transpose(0, 2, 1, 3)
    v = (xn @ wv_w.T + wv_b).reshape(B, S, H, DEP).transpose(0, 2, 1, 3)
    sc = np.einsum("bhqd,bhkd->bhqk", q, k) / np.sqrt(DEP) + mask * -1e9
    sc = sc - sc.max(-1, keepdims=True)
    e = np.exp(sc)
    a = e / e.sum(-1, keepdims=True)
    ctx = np.einsum("bhqk,bhkd->bhqd", a, v).transpose(0, 2, 1, 3).reshape(
        B, S, D)
    h = xn + ctx @ dense_w.T + dense_b
    hn = ln(h, gamma2, beta2)
    t = hn @ fc_w.T
    g = 0.5 * t * (1.0 + erf(t / np.sqrt(2.0)))
    return (hn + g @ proj_w.T).astype(np.float32)


def kernel(**inputs):
    x = np.asarray(inputs["x"], np.float32)
    mask = np.asarray(inputs["mask"], np.float32)

    causal = np.array_equal(mask, np.triu(np.ones((S, S), np.float32), k=1))
    if not causal:
        return _np_reference(**{k: np.asarray(v, np.float64 if
                                              np.asarray(v).dtype != np.int32
                                              else np.int32)
                                for k, v in inputs.items()}).reshape(B, S, D)

    if "nc" not in _cache:
        _cache["nc"] = _build_program()
    nc = _cache["nc"]

    bf16 = ml_dtypes.bfloat16
    g1 = np.asarray(inputs["gamma1"], np.float32)
    b1 = np.asarray(inputs["beta1"], np.float32)
    g2 = np.asarray(inputs["gamma2"], np.float32)
    b2 = np.asarray(inputs["beta2"], np.float32)
    dense_w = np.asarray(inputs["dense_w"], np.float32)
    dense_b = np.asarray(inputs["dense_b"], np.float32)
    fc_w = np.asarray(inputs["fc_w"], np.float32)
    proj_w = np.asarray(inputs["proj_w"], np.float32)

    xf = x.reshape(NT, D)
    shard_rows = []
    for c in range(NCORES):
        base = 512 * (c // 2) + 256 * (c % 2)
        shard_rows.append(np.concatenate(
            [base + np.arange(256), 2048 + base + np.arange(256)]))
    bcast = lambda v: np.ascontiguousarray(
        np.broadcast_to(v.astype(np.float32), (128, D)))

    # causal diagonal-block additive mask, scores_T orientation [k, q]
    md = np.zeros((4, 128, QM), np.float32)
    for i in range(4):
        kk = 128 * i + np.arange(128)[:, None]
        qq = np.arange(QM)[None, :]
        md[i][kk > qq] = NEG

    fc_eff = fc_w * g2[None, :]
    fcb = fc_w @ b2
    in_maps = []
    for c in range(NCORES):
        rows = slice(128 * c, 128 * (c + 1))
        im = {
            "x_full": xf,
            "x_shard": np.ascontiguousarray(xf[shard_rows[c]]),
            "g1b": bcast(g1), "b1b": bcast(b1 + dense_b),
            "g2b": bcast(g2), "b2b": bcast(b2),
            "dense_wt": dense_w.T.astype(bf16),
            "fc_wt": np.ascontiguousarray(
                fc_eff.T.reshape(8, 128, 32, 128).transpose(2, 1, 0, 3)
            ).astype(bf16),
            "fcb": np.ascontiguousarray(fcb.reshape(32, 128).T),
            "proj_wt": proj_w.T.astype(bf16),
            "mask_diag": md,
        }
        for nm, w, bias in (("q", np.asarray(inputs["wq_w"], np.float32),
                             np.asarray(inputs["wq_b"], np.float32)),
                            ("k", np.asarray(inputs["wk_w"], np.float32),
                             np.asarray(inputs["wk_b"], np.float32)),
                            ("v", np.asarray(inputs["wv_w"], np.float32),
                             np.asarray(inputs["wv_b"], np.float32))):
            wslice = w[rows]                     # [128, D]
            im[f"w{nm}t"] = np.ascontiguousarray(
                (wslice * g1[None, :]).T.reshape(8, 128, 128).transpose(
                    1, 0, 2)).astype(bf16)
            im[f"{nm}b"] = (bias[rows] + wslice @ b1).reshape(128, 1)
        in_maps.append(im)

    global _last_in_maps
    _last_in_maps = in_maps
    from concourse import bass_utils
    res = bass_utils.run_bass_kernel_spmd(nc, in_maps,
                                          core_ids=list(range(NCORES)))
    out = np.empty((NT, D), np.float32)
    for c in range(NCORES):
        out[shard_rows[c]] = res.results[c]["out_shard"]
    return out.reshape(B, S, D)

